# revision 2
# baseline (speedup 1.0000x reference)
"""Column-sum kernel for Trainium2: out[d] = sum_r x[r, d].

x is [8192, 4096] f32, rows sharded across 8 NeuronCores (1024 rows
each). Per-core pipeline:

The core's [1024, 4096] slice is streamed as column stripes: stripe s
loads [128, 8, W_s] (partition = row-within-block, 8 row-blocks, W_s
columns; W_s*4-byte contiguous descriptor lines keep the DMA at line
rate). Stripe s is the LAST data touching its columns, so the moment
it lands its 8 row-blocks fold to one [128, W_s] tile (adds split
between DVE and GpSimd), the 128-partition reduce closes on the PE
(ones-matmul into PSUM), PSUM copies out on ACT, and that column
slice of the output flies out — all while later stripes still stream.

Only the last (narrow) stripe's fold+close trails the stream, so the
serial tail after the final HBM byte is ~3 us instead of the ~10 us a
monolithic row-tile accumulate + final reduce pays. Host sums the 8
per-core [1, 4096] partials.
"""

import numpy as np

M_CORES = 8
ROWS, D = 8192, 4096
ROWS_PER_CORE = ROWS // M_CORES  # 1024
P = 128
J = ROWS_PER_CORE // P  # 8 row-blocks per stripe
STRIPE_W = (512, 512, 512, 512, 512, 512, 512, 256, 256)  # sum 4096

_nc_cache = None


def _build():
    import concourse.tile as tile
    from concourse import bacc, mybir

    nc = bacc.Bacc(None)
    x = nc.declare_dram_parameter(
        "x", [ROWS_PER_CORE, D], mybir.dt.float32, isOutput=False
    )
    out = nc.declare_dram_parameter("out", [1, D], mybir.dt.float32, isOutput=True)

    xr = x.rearrange("(j p) d -> p j d", p=P)  # [128, 8, 4096]

    with tile.TileContext(nc) as tc:
        with (
            tc.tile_pool(name="xpool", bufs=len(STRIPE_W)) as xpool,
            tc.tile_pool(name="vpool", bufs=len(STRIPE_W)) as vpool,
            tc.tile_pool(name="singles", bufs=1) as singles,
            tc.tile_pool(name="psum", bufs=4, space="PSUM") as psum_pool,
        ):
            ones = singles.tile([P, 1], mybir.dt.float32)
            nc.vector.memset(ones[:], 1.0)

            osb = singles.tile([1, D], mybir.dt.float32)

            # Stream all stripes up front; queues drain them in issue
            # order so stripe s arrives before stripe s+1.
            bts = []
            col = 0
            for s, W in enumerate(STRIPE_W):
                bt = xpool.tile([P, J * W], mybir.dt.float32, name=f"bt{s}", tag="bt")
                nc.sync.dma_start(
                    bt[:].rearrange("p (j w) -> p j w", j=J),
                    xr[:, :, col : col + W],
                )
                bts.append(bt)
                col += W

            # Per stripe: fold 8 row-blocks (DVE takes blocks 0-3,
            # GpSimd blocks 4-7, DVE joins), close the partition
            # reduce on the PE, copy PSUM out on ACT, write the slice.
            col = 0
            for s, W in enumerate(STRIPE_W):
                bt = bts[s].rearrange("p (j w) -> p j w", j=J)
                a = vpool.tile([P, W], mybir.dt.float32, name=f"a{s}", tag="a")
                b = vpool.tile([P, W], mybir.dt.float32, name=f"b{s}", tag="b")
                nc.vector.tensor_add(a[:], bt[:, 0, :], bt[:, 1, :])
                nc.gpsimd.tensor_add(b[:], bt[:, 4, :], bt[:, 5, :])
                nc.vector.tensor_add(a[:], a[:], bt[:, 2, :])
                nc.gpsimd.tensor_add(b[:], b[:], bt[:, 6, :])
                nc.vector.tensor_add(a[:], a[:], bt[:, 3, :])
                nc.gpsimd.tensor_add(b[:], b[:], bt[:, 7, :])
                nc.vector.tensor_add(a[:], a[:], b[:])

                ps = psum_pool.tile([1, W], mybir.dt.float32, name=f"ps{s}", tag="ps")
                nc.tensor.matmul(ps[:1, :W], ones[:], a[:], start=True, stop=True)
                nc.scalar.copy(osb[:, col : col + W], ps[:1, :W])
                nc.sync.dma_start(out[:, col : col + W], osb[:, col : col + W])
                col += W

    nc.compile()
    return nc


def _get_nc():
    global _nc_cache
    if _nc_cache is None:
        _nc_cache = _build()
    return _nc_cache


def _run(x_np: np.ndarray, **run_kwargs):
    from concourse.bass_utils import run_bass_kernel_spmd

    nc = _get_nc()
    shards = np.split(x_np, M_CORES, axis=0)
    in_maps = [{"x": np.ascontiguousarray(s)} for s in shards]
    return run_bass_kernel_spmd(nc, in_maps, list(range(M_CORES)), **run_kwargs)


def kernel(x) -> np.ndarray:
    x_np = np.ascontiguousarray(np.asarray(x), dtype=np.float32)
    assert x_np.shape == (ROWS, D), x_np.shape
    res = _run(x_np)
    partials = np.stack([r["out"][0] for r in res.results])
    return partials.sum(axis=0, dtype=np.float32)


# revision 6
# speedup vs baseline: 1.0410x; 1.0410x over previous
"""Column-sum kernel for Trainium2: out[d] = sum_r x[r, d].

x is [8192, 4096] f32, rows sharded across 8 NeuronCores (1024 rows
each). The host pre-packs each core's [1024, 4096] slice into
column-stripe-major order: stripe s is the contiguous [1024, W_s]
block of columns [c_s, c_s + W_s). On device each stripe loads as ONE
plain 2D DMA — partition p takes rows [8p, 8p+8), a single contiguous
32*W_s-byte line — so descriptors stay fat (>= 2 KiB) and issue cost
stays flat regardless of stripe width.

Stripe s is the last data touching its columns, so the moment it
lands, its 8 row-slots fold to [128, W_s] (adds split between DVE and
GpSimd), the 128-partition reduce closes on the PE (ones-matmul into
PSUM), ACT copies PSUM out, and that column slice of the output is
written — all while later stripes still stream. Stripe widths taper
(..., 320, 128, 64) so only a tiny fold+close trails the final HBM
byte, instead of the ~10 us a monolithic row-tile accumulate + final
[128, 4096] reduce pays. Host sums the 8 per-core [1, 4096] partials.
"""

import numpy as np

M_CORES = 8
ROWS, D = 8192, 4096
ROWS_PER_CORE = ROWS // M_CORES  # 1024
P = 128
J = ROWS_PER_CORE // P  # 8 row-slots per partition
STRIPE_W = (512, 512, 512, 512, 512, 512, 512, 320, 128, 64)  # sum 4096

assert sum(STRIPE_W) == D

_nc_cache = None


def _build():
    import concourse.tile as tile
    from concourse import bacc, mybir

    nc = bacc.Bacc(None)
    x = nc.declare_dram_parameter(
        "x", [ROWS_PER_CORE * D], mybir.dt.float32, isOutput=False
    )
    out = nc.declare_dram_parameter("out", [1, D], mybir.dt.float32, isOutput=True)

    with tile.TileContext(nc) as tc:
        with (
            tc.tile_pool(name="xpool", bufs=8) as xpool,
            tc.tile_pool(name="singles", bufs=1) as singles,
            tc.tile_pool(name="psum", bufs=4, space="PSUM") as psum_pool,
        ):
            ones = singles.tile([P, 1], mybir.dt.float32)
            nc.vector.memset(ones[:], 1.0)

            osb = singles.tile([1, D], mybir.dt.float32)

            # Stream all stripes up front; queues drain them in issue
            # order so stripe s arrives before stripe s+1.
            bts = []
            off = 0
            for s, W in enumerate(STRIPE_W):
                bt = xpool.tile([P, J * W], mybir.dt.float32, name=f"bt{s}", tag="bt")
                nc.sync.dma_start(
                    bt[:],
                    x[off : off + ROWS_PER_CORE * W].rearrange("(p f) -> p f", p=P),
                )
                bts.append(bt)
                off += ROWS_PER_CORE * W

            # Per stripe: fold the 8 row-slots in place (DVE folds
            # slots 1-3 into slot 0, GpSimd slots 5-7 into slot 4, DVE
            # joins), close the partition reduce on the PE, copy PSUM
            # out on ACT, write the slice.
            col = 0
            for s, W in enumerate(STRIPE_W):
                bt = bts[s]
                a = bt[:, 0 * W : 1 * W]
                b = bt[:, 4 * W : 5 * W]
                nc.vector.tensor_add(a, a, bt[:, 1 * W : 2 * W])
                nc.gpsimd.tensor_add(b, b, bt[:, 5 * W : 6 * W])
                nc.vector.tensor_add(a, a, bt[:, 2 * W : 3 * W])
                nc.gpsimd.tensor_add(b, b, bt[:, 6 * W : 7 * W])
                nc.vector.tensor_add(a, a, bt[:, 3 * W : 4 * W])
                nc.gpsimd.tensor_add(b, b, bt[:, 7 * W : 8 * W])
                nc.vector.tensor_add(a, a, b)

                ps = psum_pool.tile([1, W], mybir.dt.float32, name=f"ps{s}", tag="ps")
                nc.tensor.matmul(ps[:1, :W], ones[:], a, start=True, stop=True)
                nc.scalar.copy(osb[:, col : col + W], ps[:1, :W])
                nc.sync.dma_start(out[:, col : col + W], osb[:, col : col + W])
                col += W

    nc.compile()
    return nc


def _get_nc():
    global _nc_cache
    if _nc_cache is None:
        _nc_cache = _build()
    return _nc_cache


def _pack_core(xc: np.ndarray) -> np.ndarray:
    """Stripe-major repack of one core's [1024, 4096] slice."""
    parts = []
    col = 0
    for W in STRIPE_W:
        parts.append(np.ascontiguousarray(xc[:, col : col + W]).ravel())
        col += W
    return np.concatenate(parts)


def _run(x_np: np.ndarray, **run_kwargs):
    from concourse.bass_utils import run_bass_kernel_spmd

    nc = _get_nc()
    shards = np.split(x_np, M_CORES, axis=0)
    in_maps = [{"x": _pack_core(s)} for s in shards]
    return run_bass_kernel_spmd(nc, in_maps, list(range(M_CORES)), **run_kwargs)


def kernel(x) -> np.ndarray:
    x_np = np.ascontiguousarray(np.asarray(x), dtype=np.float32)
    assert x_np.shape == (ROWS, D), x_np.shape
    res = _run(x_np)
    partials = np.stack([r["out"][0] for r in res.results])
    return partials.sum(axis=0, dtype=np.float32)


# revision 8
# speedup vs baseline: 1.2576x; 1.2081x over previous
"""Column-sum kernel for Trainium2: out[d] = sum_r x[r, d].

x is [8192, 4096] f32, rows sharded across 8 NeuronCores (1024 rows
each). Per-core pipeline:

- All loads are gpsimd (SWDGE) DMAs that cast f32 -> bf16 inline, so
  the PE can reduce with single-pass bf16 ones-matmuls instead of the
  2-pass fp32 LOW_HIGH path, and SBUF writes halve.
- Rows 0..895 load as seven [128, 4096] row tiles. As tile k lands,
  eight [128, 512] chunk matmuls accumulate it into per-chunk PSUM
  banks (ones^T @ chunk, start on k==0). No DVE/GpSimd fold at all --
  the PE does the whole reduction, ~0.5 us per chunk matmul.
- Rows 896..1023 load as eight [128, 512] column stripes. Stripe c is
  the LAST data touching PSUM chunk c, so its close matmul, the ACT
  copy to SBUF, and chunk c's output DMA all fire as soon as it lands
  while later stripes still stream. Only the last stripe's
  matmul+copy+store trails the final HBM byte (~2 us, vs ~10 us for a
  monolithic fold + final [128, 4096] fp32 reduce).

Accumulation stays fp32 (PSUM); only the inputs round to bf16, so the
column-sum error is ~0.3% -- far inside the 2e-2 gate. Host sums the
8 per-core [1, 4096] f32 partials.
"""

import numpy as np

M_CORES = 8
ROWS, D = 8192, 4096
ROWS_PER_CORE = ROWS // M_CORES  # 1024
P = 128
NTILE = 7  # rows 0..895 as [128, 4096] row tiles
NCHUNK = 512  # PSUM bank: fp32 [1, 512]; 8 chunks cover 4096 cols
NSTRIPE = D // NCHUNK  # 8 stripes for rows 896..1023

_nc_cache = None


def _build():
    import concourse.tile as tile
    from concourse import bacc, mybir

    nc = bacc.Bacc(None)
    x = nc.declare_dram_parameter(
        "x", [ROWS_PER_CORE, D], mybir.dt.float32, isOutput=False
    )
    out = nc.declare_dram_parameter("out", [1, D], mybir.dt.float32, isOutput=True)

    with tile.TileContext(nc) as tc:
        with (
            tc.tile_pool(name="xpool", bufs=1) as xpool,
            tc.tile_pool(name="psum", bufs=1, space="PSUM") as psum_pool,
        ):
            ones = xpool.tile([P, 1], mybir.dt.bfloat16)
            nc.vector.memset(ones[:], 1.0)

            osb = xpool.tile([1, D], mybir.dt.float32)

            # Stream everything up front (single SWDGE queue keeps
            # arrival in issue order): 7 casting row-tile loads, then
            # 8 casting stripe loads for the last 128 rows.
            bts = []
            for k in range(NTILE):
                bt = xpool.tile([P, D], mybir.dt.bfloat16, name=f"bt{k}")
                nc.gpsimd.dma_start(bt[:], x[k * P : (k + 1) * P, :])
                bts.append(bt)
            sts = []
            for c in range(NSTRIPE):
                st = xpool.tile([P, NCHUNK], mybir.dt.bfloat16, name=f"st{c}")
                nc.gpsimd.dma_start(
                    st[:], x[NTILE * P :, c * NCHUNK : (c + 1) * NCHUNK]
                )
                sts.append(st)

            pss = [
                psum_pool.tile([1, NCHUNK], mybir.dt.float32, name=f"ps{c}")
                for c in range(NSTRIPE)
            ]

            # PE accumulates tile k into all 8 chunk banks as it lands.
            for k in range(NTILE):
                for c in range(NSTRIPE):
                    nc.tensor.matmul(
                        pss[c][:1, :],
                        ones[:],
                        bts[k][:, c * NCHUNK : (c + 1) * NCHUNK],
                        start=(k == 0),
                        stop=False,
                    )

            # Stripe c closes chunk c the moment it lands; its columns
            # fly out while later stripes still stream.
            for c in range(NSTRIPE):
                nc.tensor.matmul(
                    pss[c][:1, :], ones[:], sts[c][:], start=False, stop=True
                )
                nc.scalar.copy(osb[:, c * NCHUNK : (c + 1) * NCHUNK], pss[c][:1, :])
                nc.sync.dma_start(
                    out[:, c * NCHUNK : (c + 1) * NCHUNK],
                    osb[:, c * NCHUNK : (c + 1) * NCHUNK],
                )

    nc.compile()
    return nc


def _get_nc():
    global _nc_cache
    if _nc_cache is None:
        _nc_cache = _build()
    return _nc_cache


def _run(x_np: np.ndarray, **run_kwargs):
    from concourse.bass_utils import run_bass_kernel_spmd

    nc = _get_nc()
    shards = np.split(x_np, M_CORES, axis=0)
    in_maps = [{"x": np.ascontiguousarray(s)} for s in shards]
    return run_bass_kernel_spmd(nc, in_maps, list(range(M_CORES)), **run_kwargs)


def kernel(x) -> np.ndarray:
    x_np = np.ascontiguousarray(np.asarray(x), dtype=np.float32)
    assert x_np.shape == (ROWS, D), x_np.shape
    res = _run(x_np)
    partials = np.stack([r["out"][0] for r in res.results])
    return partials.sum(axis=0, dtype=np.float32)
